# revision 1
# baseline (speedup 1.0000x reference)
"""Trainium2 Bass kernel for nn_DiscriminatorModel (8-layer MLP with
LayerNorm+LeakyReLU, 524288x128 input, data-parallel over 8 NeuronCores).

Algorithm (validated in numpy to ~7e-4 relative absmax vs the jax reference):
  - Mean-centering of each LayerNorm is folded into the weights host-side:
    Wc_l = W_l @ (I - 11^T/d)  => matmul output is already centered.
  - The per-row rsqrt(var+eps) scales commute through LeakyReLU and the
    following matmul, so they are never applied per-layer; only the variances
    of layers 6 and 7 matter to fp32 precision:
        E8 = v7 + eps*v6  (+O(eps^2)),  out = (a7 @ W8) / sqrt(E8) + b8
  - fp32-grade precision via fp16 multi-word matmuls (3 terms):
        q = Whi@ahi + Whi@alo + Wlo@ahi,  fp32 PSUM accumulate.
  - Activations are packed "feature-major": 128 partitions = c blocks x dout
    features, rows along the free dim. LeakyReLU+gamma runs as ONE ScalarE
    activation instruction per tile (Prelu, per-partition scale, alpha=0.2;
    note Lrelu ignores alpha on this table build - Prelu honors it)
    reading PSUM directly. hi/lo split on VectorE/GpSimd.

Requires all LayerNorm beta == 0 (true for the reference inputs); otherwise
falls back to a numpy forward pass.
"""

import numpy as np

EPS = 1e-5
SLOPE = 0.2
DIMS = [128, 32, 64, 32, 16, 8, 4, 2]
N_CORES = 8
ROWS = 524288
RPC = ROWS // N_CORES        # 65536 rows per core
R_ST = 8192                  # rows per supertile
N_ST = RPC // R_ST           # 8 supertiles per core
F16 = np.float16

_CACHE = {}


def _lrelu(x):
    return np.where(x > 0, x, SLOPE * x).astype(np.float32)


def _center(W):
    d = W.shape[1]
    return (W.astype(np.float64) @ (np.eye(d) - 1.0 / d)).astype(np.float32)


def _split(a):
    hi = a.astype(F16)
    lo = (a.astype(np.float32) - hi.astype(np.float32)).astype(F16)
    return hi, lo


def _blockdiag(W, c):
    din, dout = W.shape
    out = np.zeros((c * din, c * dout), W.dtype)
    for b in range(c):
        out[b * din:(b + 1) * din, b * dout:(b + 1) * dout] = W
    return out


def _transition_stat(W, c_in):
    """Parity-interleaved stationary for a c_in -> 2*c_in packing transition.

    Two stats (par=0,1), each [128, 128]: out col m = blk_out*w + f where
    w = 128/(2*c_in) per-block output width; nonzero iff blk_out % 2 == par,
    source block g = blk_out // 2 maps rows g*din..(g+1)*din <- W[:, f].
    """
    din, dout = W.shape
    w = 128 // (2 * c_in)
    assert w == dout
    stats = []
    for par in range(2):
        S = np.zeros((128, 128), W.dtype)
        for m in range(128):
            blk_out, f = divmod(m, w)
            if blk_out % 2 != par:
                continue
            g = blk_out // 2
            S[g * din:(g + 1) * din, m] = W[:, f]
        stats.append(S)
    return stats


def _var_stats(dout6, dout7):
    # V6 par-stats: s6 is 32-packed (32 blocks x 4 feats); v6' is 64 blocks.
    V6 = []
    for par in range(2):
        S = np.zeros((128, 64), np.float32)
        for m in range(64):
            if m % 2 != par:
                continue
            g = m // 2
            S[g * dout6:(g + 1) * dout6, m] = 1.0 / dout6
        V6.append(S)
    V7 = np.zeros((128, 64), np.float32)
    for m in range(64):
        V7[m * dout7:(m + 1) * dout7, m] = 1.0 / dout7
    return V6[0], V6[1], V7


def _numpy_forward(inp):
    h = np.asarray(inp["x"], np.float32)
    for i in range(7):
        W = np.asarray(inp[f"W{i+1}"], np.float32)
        g = np.asarray(inp[f"g{i+1}"], np.float32)
        b = np.asarray(inp[f"bt{i+1}"], np.float32)
        h = h @ W
        m = h.mean(-1, keepdims=True)
        v = np.square(h - m).mean(-1, keepdims=True)
        h = (h - m) / np.sqrt(v + EPS) * g + b
        h = _lrelu(h)
    return (h @ np.asarray(inp["W8"], np.float32)
            + np.asarray(inp["b8"], np.float32)).astype(np.float32)


def _build_consts(inp):
    """Host-side weight prep. Returns dict of DRAM const arrays."""
    Wc = [_center(np.asarray(inp[f"W{l}"], np.float32)) for l in range(1, 8)]
    g = [np.asarray(inp[f"g{l}"], np.float32) for l in range(1, 8)]
    W8 = np.asarray(inp["W8"], np.float32)

    cols = {}
    def add(name, arr32, pair=True):
        if pair:
            hi, lo = _split(arr32)
            cols[name + "h"], cols[name + "l"] = hi, lo
        else:
            cols[name] = arr32.astype(F16)

    add("s1", Wc[0])                                   # [128, 32]
    # L2 row-tiled: blockdiag2(Wc2) [64,128] stacked twice -> [128,128]
    bd2 = _blockdiag(Wc[1], 2)
    add("s2", np.vstack([bd2, bd2]))
    add("s3", _blockdiag(Wc[2], 2))                    # [128, 64]
    for l, c_in in ((4, 4), (5, 8), (6, 16), (7, 32)):
        t0, t1 = _transition_stat(Wc[l - 1], c_in)
        add(f"t{l}a", t0)
        add(f"t{l}b", t1)
    add("s8", _blockdiag(W8, 64))                      # [128, 64]

    # pack all fp16 stationaries into one [128, T] array; remember offsets
    order = sorted(cols.keys())
    offs, total = {}, 0
    for k in order:
        offs[k] = total
        total += cols[k].shape[1]
    wpack = np.zeros((128, total), F16)
    for k in order:
        wpack[:, offs[k]:offs[k] + cols[k].shape[1]] = cols[k]

    V6a, V6b, V7 = _var_stats(DIMS[6], DIMS[7])
    vpack = np.concatenate([V6a, V6b, V7], axis=1).astype(np.float32)

    # per-partition gamma vectors matching each layer's output packing
    gv = np.zeros((128, 8), np.float32)
    widths = [32, 64, 32, 16, 8, 4, 2]
    for i in range(7):
        gv[:, i] = np.tile(g[i], 128 // widths[i])
    return wpack, offs, gv, vpack



def _split_multi_waits(nc):
    """Walrus build limit: <=1 sync wait per instruction. Hoist extras onto
    same-engine NOPs inserted just before the instruction."""
    import concourse.mybir as mybir
    import bass_rust
    cnt = 0
    for f in nc.m.functions:
        for blk in f.blocks:
            newlist = []
            for inst in blk.instructions:
                si = inst.sync_info
                waits = list(si.on_wait) if si is not None and si.on_wait else []
                if len(waits) > 1:
                    for w in waits[:-1]:
                        nop = mybir.InstNoOp(name=f"waitnop_{cnt}", ins=[], outs=[])
                        cnt += 1
                        nop.engine = inst.engine
                        nop.sync_info = bass_rust.SyncInfo(on_wait=[w], on_update=[])
                        newlist.append(nop)
                    inst.sync_info = bass_rust.SyncInfo(
                        on_wait=[waits[-1]], on_update=list(si.on_update))
                newlist.append(inst)
            blk.instructions = newlist
    return cnt


def _build_program(offs, wpack_cols, b8_val):
    import concourse.bass as bass
    import concourse.mybir as mybir
    from concourse.tile import TileContext
    from contextlib import ExitStack

    # this walrus build rejects >1 sync wait on the tail Drain; split them
    import bass_rust
    from concourse.tile import TileContext as _TC
    from concourse.vector_clock import ScopedClock

    def _patched_drain(self, tick_clock, wait_clock):
        probe = self.nc.sync.nop()
        wait_clock.add_sem_waits(probe.ins,
                                 ScopedClock({None: tick_clock.global_clock}))
        si = probe.ins.sync_info
        waits = list(si.on_wait) if si is not None else []
        upd = list(si.on_update) if si is not None else []
        probe.ins.sync_info = bass_rust.SyncInfo(on_wait=waits[:1], on_update=upd)
        for w in waits[1:]:
            nop = self.nc.sync.nop()
            nop.ins.sync_info = bass_rust.SyncInfo(on_wait=[w], on_update=[])
        self.nc.sync.drain()
        self.nc.all_engine_barrier()
        assert self.sems is not None
        popped = self.nc._tile_sem_poison_stack.pop()
        assert popped is self._sem_poison
        self.nc.clear_and_free_semaphores(list(self.sems.allocated().values()))
        self.nc.all_engine_barrier()

    _TC._drain_and_barrier = _patched_drain

    f16, f32 = mybir.dt.float16, mybir.dt.float32
    AF = mybir.ActivationFunctionType
    OP = mybir.AluOpType

    nc = bass.Bass(trn_type="TRN2", num_swdge_queues=4)
    xhi_d = nc.dram_tensor("xhi", [128, RPC], f16, kind="ExternalInput")
    xlo_d = nc.dram_tensor("xlo", [128, RPC], f16, kind="ExternalInput")
    wp_d = nc.dram_tensor("wpack", [128, wpack_cols], f16, kind="ExternalInput")
    gv_d = nc.dram_tensor("gv", [128, 8], f32, kind="ExternalInput")
    vp_d = nc.dram_tensor("vpack", [128, 192], f32, kind="ExternalInput")
    out_d = nc.dram_tensor("out", [N_ST * 64, R_ST // 64], f32,
                           kind="ExternalOutput")

    with TileContext(nc) as tc:
        with ExitStack() as ctx:
            const = ctx.enter_context(tc.tile_pool(name="const", bufs=1))
            wp = const.tile([128, wpack_cols], f16)
            nc.sync.dma_start(wp[:, :], wp_d[:, :])
            gv = const.tile([128, 8], f32)
            nc.sync.dma_start(gv[:, :], gv_d[:, :])
            vpk = const.tile([128, 192], f32)
            nc.sync.dma_start(vpk[:, :], vp_d[:, :])

            def W(name):
                return wp[:, offs[name]:offs[name] + _WCOLS[name]]

            xp = ctx.enter_context(tc.tile_pool(name="xp", bufs=2))
            ap = ctx.enter_context(tc.tile_pool(name="ap", bufs=2))
            fin = ctx.enter_context(tc.tile_pool(name="fin", bufs=2 * N_ST))
            up = ctx.enter_context(tc.tile_pool(name="up", bufs=2, space="PSUM"))
            vp = ctx.enter_context(tc.tile_pool(name="vp", bufs=3, space="PSUM"))

            def mm(out, lhsT, rhs, start, stop, tp=None):
                # matmul output must fit one PSUM bank: 512 fp32 columns
                n = out.shape[1]
                for o in range(0, n, 512):
                    e = min(o + 512, n)
                    nc.tensor.matmul(out[:, o:e], lhsT, rhs[:, o:e],
                                     start=start, stop=stop, tile_position=tp)

            ysbs, e8sbs = [], []

            for st in range(N_ST):
                x0 = st * R_ST
                xh = []
                xl = []
                dma_engs = [nc.sync, nc.gpsimd, nc.scalar, nc.gpsimd]
                for k in range(2):
                    xht = xp.tile([128, 4096], f16, name=f"xh{k}")
                    dma_engs[2 * k].dma_start(
                        xht[:, :],
                        xhi_d[:, x0 + 4096 * k:x0 + 4096 * (k + 1)])
                    xh.append(xht)
                    xlt = xp.tile([128, 4096], f16, name=f"xl{k}")
                    dma_engs[2 * k + 1].dma_start(
                        xlt[:, :],
                        xlo_d[:, x0 + 4096 * k:x0 + 4096 * (k + 1)])
                    xl.append(xlt)

                def act_split(u, gcol, width, hi, lo, col0, eng):
                    """ACT Lrelu (PSUM->SBUF fp32), then hi/lo fp16 split."""
                    n = u.shape[1]
                    af = ap.tile([128, 1024], f32, name="af", tag="af", bufs=4)
                    afv = af[:, :n]
                    nc.scalar.activation(afv, u[:, :], AF.Prelu,
                                         bias=0.0, scale=gv[:, gcol:gcol + 1],
                                         alpha=SLOPE)
                    nc.vector.tensor_copy(hi[:, col0:col0 + n], afv)
                    eng.tensor_tensor(lo[:, col0:col0 + n], afv,
                                      hi[:, col0:col0 + n], OP.subtract)

                # ---- L1: u1 [128, 2048] (c=4), 2 psum chunks
                a1h = ap.tile([128, 2048], f16)
                a1l = ap.tile([128, 2048], f16)
                for c in range(2):
                    u = up.tile([128, 1024], f32, name="u", tag="u")
                    # term-outer order: adjacent mms hit different col-groups
                    # so their LDWEIGHTS overlap in-flight matmuls
                    for t in range(3):
                        for b in range(4):
                            rh = xh[b // 2][:, (b % 2) * 2048 + 1024 * c:][:, :1024]
                            rl = xl[b // 2][:, (b % 2) * 2048 + 1024 * c:][:, :1024]
                            S = W("s1h") if t < 2 else W("s1l")
                            r = rh if t != 1 else rl
                            mm(u[32 * b:32 * (b + 1), :], S, r,
                               start=(t == 0), stop=(t == 2), tp=(0, 32 * b))
                    act_split(u, 0, 32, a1h, a1l, 1024 * c, nc.vector)

                # ---- L2 row-tiled: two tensors u2_q [128, 2048]
                a2h = [ap.tile([128, 2048], f16, name=f"a2h{q}") for q in range(2)]
                a2l = [ap.tile([128, 2048], f16, name=f"a2l{q}") for q in range(2)]
                s2hs = [wp[:, offs["s2h"]:offs["s2h"] + 128][64 * q:64 * (q + 1), :]
                        for q in range(2)]
                s2ls = [wp[:, offs["s2l"]:offs["s2l"] + 128][64 * q:64 * (q + 1), :]
                        for q in range(2)]
                for c in range(2):
                    us = [up.tile([128, 1024], f32, name="u", tag="u")
                          for _ in range(2)]
                    for t in range(3):
                        for q in range(2):
                            rh = a1h[64 * q:64 * (q + 1), 1024 * c:1024 * (c + 1)]
                            rl = a1l[64 * q:64 * (q + 1), 1024 * c:1024 * (c + 1)]
                            S = s2hs[q] if t < 2 else s2ls[q]
                            r = rh if t != 1 else rl
                            mm(us[q][:, :], S, r, start=(t == 0),
                               stop=(t == 2), tp=(64 * q, 0))
                    for q in range(2):
                        act_split(us[q], 1, 64, a2h[q], a2l[q], 1024 * c,
                                  nc.vector)

                # ---- L3 col-tiled: u3 [128, 2048] (c=4)
                a3h = ap.tile([128, 2048], f16)
                a3l = ap.tile([128, 2048], f16)
                for c in range(2):
                    u = up.tile([128, 1024], f32, name="u", tag="u")
                    for t in range(3):
                        for q in range(2):
                            rh = a2h[q][:, 1024 * c:1024 * (c + 1)]
                            rl = a2l[q][:, 1024 * c:1024 * (c + 1)]
                            S = W("s3h") if t < 2 else W("s3l")
                            r = rh if t != 1 else rl
                            mm(u[64 * q:64 * (q + 1), :], S, r,
                               start=(t == 0), stop=(t == 2), tp=(0, 64 * q))
                    act_split(u, 2, 32, a3h, a3l, 1024 * c, nc.vector)

                # ---- L4..L7: parity transitions, halving free size
                prev_h, prev_l = a3h, a3l
                n_prev = 2048
                s6 = s7 = None
                for li, l in enumerate((4, 5, 6, 7)):
                    n = n_prev // 2
                    u = up.tile([128, 1024], f32, name="u", tag="u")
                    uv = u[:, :n]
                    first = True
                    for par, suf in ((0, "a"), (1, "b")):
                        rh = prev_h[:, par * n:(par + 1) * n]
                        rl = prev_l[:, par * n:(par + 1) * n]
                        th, tl = W(f"t{l}{suf}h"), W(f"t{l}{suf}l")
                        mm(uv, th, rh, start=first, stop=False)
                        mm(uv, th, rl, start=False, stop=False)
                        mm(uv, tl, rh, start=False,
                                         stop=(par == 1))
                        first = False
                    nh = ap.tile([128, n], f16, name=f"a{l}h")
                    nl = ap.tile([128, n], f16, name=f"a{l}l")
                    if l == 6:
                        s6 = ap.tile([128, 256], f32)
                        nc.scalar.activation(s6[:, :], uv, AF.Square)
                    if l == 7:
                        s7 = ap.tile([128, 128], f32)
                        nc.scalar.activation(s7[:, :], uv, AF.Square)
                    act_split(u[:, :n], l - 1, 128 // (2 ** (li + 3)), nh, nl,
                              0, nc.vector)
                    prev_h, prev_l, n_prev = nh, nl, n

                # ---- variances
                v6t = vp.tile([64, 128], f32, name="v6t", tag="v")
                mm(v6t[:, :], vpk[:, 0:64], s6[:, 0:128],
                   start=True, stop=False)
                mm(v6t[:, :], vpk[:, 64:128], s6[:, 128:256],
                   start=False, stop=True)
                v7t = vp.tile([64, 128], f32, name="v7t", tag="v")
                mm(v7t[:, :], vpk[:, 128:192], s7[:, :],
                   start=True, stop=True)

                # ---- L8: y = a7 @ blockdiag64(W8)
                yt = vp.tile([64, 128], f32, name="yt", tag="v")
                mm(yt[:, :], W("s8h"), prev_h[:, :],
                                 start=True, stop=False)
                mm(yt[:, :], W("s8h"), prev_l[:, :],
                                 start=False, stop=False)
                mm(yt[:, :], W("s8l"), prev_h[:, :],
                                 start=False, stop=True)

                # ---- stash y and E8 = v7 + eps*v6 (sqrt deferred)
                v7sb = fin.tile([64, 128], f32, name="v7sb", tag="v7sb")
                nc.scalar.copy(v7sb[:, :], v7t[:, :])
                ysb = fin.tile([64, 128], f32, name="ysb", tag="ysb")
                nc.scalar.copy(ysb[:, :], yt[:, :])
                e8 = fin.tile([64, 128], f32, name="e8", tag="e8")
                nc.vector.scalar_tensor_tensor(e8[:, :], v6t[:, :], EPS,
                                               v7sb[:, :], OP.mult, OP.add)
                ysbs.append(ysb)
                e8sbs.append(e8)

            # ---- final: out = y / sqrt(E8) + b8 (one Sqrt table-load)
            for st in range(N_ST):
                sq = fin.tile([64, 128], f32, name="sq", tag="sq", bufs=2)
                nc.scalar.activation(sq[:, :], e8sbs[st][:, :], AF.Sqrt)
                rinv = fin.tile([64, 128], f32, name="rinv", tag="rinv", bufs=2)
                nc.vector.reciprocal(rinv[:, :], sq[:, :])
                osb = fin.tile([64, 128], f32, name="osb", tag="osb", bufs=2)
                nc.vector.tensor_tensor(osb[:, :], ysbs[st][:, :], rinv[:, :],
                                        OP.mult)
                nc.vector.tensor_scalar(osb[:, :], osb[:, :], b8_val,
                                        None, OP.add)
                nc.sync.dma_start(out_d[st * 64:(st + 1) * 64, :], osb[:, :])
    _split_multi_waits(nc)
    return nc


_WCOLS = {}


def kernel(**inputs):
    for l in range(1, 8):
        if np.abs(np.asarray(inputs[f"bt{l}"], np.float32)).max() > 0:
            return _numpy_forward(inputs)

    wpack, offs, gv, vpack = _build_consts(inputs)
    global _WCOLS
    _WCOLS = {"s1h": 32, "s1l": 32, "s2h": 128, "s2l": 128, "s3h": 64,
              "s3l": 64, "s8h": 64, "s8l": 64, "v6a": 64, "v6b": 64, "v7": 64}
    for l in range(4, 8):
        for suf in ("a", "b"):
            _WCOLS[f"t{l}{suf}h"] = 128
            _WCOLS[f"t{l}{suf}l"] = 128

    x = np.asarray(inputs["x"], np.float32)
    xT = np.ascontiguousarray(x.T)               # [128, 524288]
    xhi = xT.astype(F16)
    xlo = (xT - xhi.astype(np.float32)).astype(F16)
    b8 = np.asarray(inputs["b8"], np.float32).reshape(1, 1)

    nc = _build_program(offs, wpack.shape[1], float(b8[0, 0]))

    in_maps = []
    for c in range(N_CORES):
        s = slice(c * RPC, (c + 1) * RPC)
        in_maps.append({
            "xhi": np.ascontiguousarray(xhi[:, s]),
            "xlo": np.ascontiguousarray(xlo[:, s]),
            "wpack": wpack, "gv": gv, "vpack": vpack,
        })

    from concourse.bass_utils import run_bass_kernel_spmd
    res = run_bass_kernel_spmd(nc, in_maps, core_ids=list(range(N_CORES)))

    out = np.empty((ROWS, 1), np.float32)
    for c in range(N_CORES):
        out[c * RPC:(c + 1) * RPC, 0] = res.results[c]["out"].reshape(-1)
    return out



# revision 12
# speedup vs baseline: 1.1392x; 1.1392x over previous
"""Trainium2 Bass kernel for nn_DiscriminatorModel (8-layer MLP with
LayerNorm+LeakyReLU, 524288x128 input, data-parallel over 8 NeuronCores).

Numerics (validated in numpy to ~8e-4 relative absmax vs the jax reference):
  - LayerNorm mean-centering folded into weights host-side (Wc = W @ (I-1/d)).
  - Per-row rsqrt(var+eps) scales telescope through LeakyReLU/matmuls;
    only v6, v7 matter: out = (a7 @ W8) / sqrt(v7 + eps*v6) + b8.
  - fp32-grade activations via fp16 hi/lo pairs, 3-term matmuls
    (Sh@ah + Sh@al + Sl@ah) accumulated in fp32 PSUM. Squares for v6/v7
    kept fp32 (fp16 underflows: per-row scale drifts to ~1e-5).

Execution layout (new vs prior baseline): every layer runs as 4 concurrent
PE-tile matmul slots (disjoint 32x32 tile groups via tile_position), so
TensorE streams ~4 cols/cycle instead of 1. Activations are packed
feature-major [c blocks x dout feats = 128 partitions, rows along free dim]
with "concat" packing transitions (block b' = g + c*h) that keep every
matmul output 32-partition aligned. Drain work (Prelu + fp16 hi cast + lo
subtract) is spread across ScalarE / VectorE / GpSimd.
"""

import numpy as np

EPS = 1e-5
SLOPE = 0.2
DIMS = [128, 32, 64, 32, 16, 8, 4, 2]
N_CORES = 8
ROWS = 524288
RPC = ROWS // N_CORES        # 65536 rows per core
R_ST = 8192                  # rows per supertile
N_ST = RPC // R_ST           # 8 supertiles per core
F16 = np.float16

# ---------------------------------------------------------------- layout ---


def _l1_slots():
    return [dict(kr=(0, 128), mr=(32 * b, 32 * b + 32),
                 rhs=(2048 * b, 2048 * (b + 1)), oc=(0, 2048),
                 tp=(0, 32 * b)) for b in range(4)]


def _l2_slots():
    out = []
    for b in range(4):
        s, p = b % 2, b // 2
        out.append(dict(kr=(32 * b, 32 * b + 32), mr=(64 * p, 64 * p + 64),
                        rhs=(0, 2048), oc=(2048 * s, 2048 * s + 2048),
                        tp=(32 * b, 64 * p)))
    return out


def _l3_slots():
    out = []
    for q in range(4):
        p, s = q % 2, q // 2
        out.append(dict(kr=(64 * p, 64 * p + 64), mr=(32 * q, 32 * q + 32),
                        rhs=(2048 * s, 2048 * s + 2048), oc=(0, 2048),
                        tp=(64 * p, 32 * q)))
    return out


def _trans_slots(n_in):
    out = []
    for Q in range(2):
        for h in range(2):
            out.append(dict(kr=(64 * Q, 64 * Q + 64),
                            mr=(32 * (Q + 2 * h), 32 * (Q + 2 * h) + 32),
                            rhs=(h * n_in // 2, (h + 1) * n_in // 2),
                            oc=(0, n_in // 2),
                            tp=(64 * Q, 32 * (Q + 2 * h))))
    return out


LAYER_SLOTS = {1: _l1_slots(), 2: _l2_slots(), 3: _l3_slots(),
               4: _trans_slots(2048), 5: _trans_slots(1024),
               6: _trans_slots(512), 7: _trans_slots(256)}
N_OUT = {1: 2048, 2: 4096, 3: 2048, 4: 1024, 5: 512, 6: 256, 7: 128}
# psum window order per layer (1024-col windows; L2's ordered so adjacent
# windows use disjoint PE tiles)
WINDOWS = {1: (0, 1024), 2: (0, 2048, 1024, 3072), 3: (0, 1024),
           4: (0,), 5: (0,), 6: (0,), 7: (0,)}

# drain engine assignment (tuning knobs): prelu 's'calar | 'v'ector
# (DVE scalar_tensor_tensor cannot read two PSUM operands -> Prelu from PSUM
# must run on ScalarE)
PRELU_ENG = {1: 's', 2: 's', 3: 's', 4: 's', 5: 's', 6: 's', 7: 's'}
LO_ENG = {1: 'v', 2: 'g', 3: 'v', 4: 'v', 5: 'v', 6: 'v'}


def _build_stationary(l, W):
    din, dout = W.shape
    S = np.zeros((128, 128), np.float32)
    for sl in LAYER_SLOTS[l]:
        k0, k1 = sl["kr"]
        m0, m1 = sl["mr"]
        nblk = (k1 - k0) // din
        for i in range(nblk):
            S[k0 + i * din:k0 + (i + 1) * din,
              m0 + i * dout:m0 + (i + 1) * dout] = W
    return S


def _build_v6():
    V = np.zeros((128, 64), np.float32)
    for h in range(2):
        for n7 in range(32 * h, 32 * h + 32):
            n6 = n7 - 32 * h
            V[4 * n6:4 * n6 + 4, n7] = 0.25
    return V


def _build_v7():
    V = np.zeros((128, 64), np.float32)
    for n7 in range(64):
        V[2 * n7:2 * n7 + 2, n7] = 0.5
    return V


def _build_s8(W8):
    S = np.zeros((128, 64), np.float32)
    for n in range(64):
        S[2 * n:2 * n + 2, n] = W8[:, 0]
    return S


def _rowmap7():
    rm1 = np.arange(R_ST).reshape(4, 2048)
    rm2 = np.zeros((2, 4096), np.int64)
    for p in range(2):
        for s in range(2):
            rm2[p, 2048 * s:2048 * (s + 1)] = rm1[s + 2 * p]
    rm3 = np.zeros((4, 2048), np.int64)
    for q in range(4):
        rm3[q] = rm2[q % 2, 2048 * (q // 2):2048 * (q // 2) + 2048]
    cur = rm3
    for c_in in (4, 8, 16, 32):
        n_in = cur.shape[1]
        nxt = np.zeros((2 * c_in, n_in // 2), np.int64)
        for Q in range(2):
            for h in range(2):
                for i in range(c_in // 2):
                    nxt[(c_in // 2) * (Q + 2 * h) + i] = \
                        cur[(c_in // 2) * Q + i,
                            (n_in // 2) * h:(n_in // 2) * (h + 1)]
        cur = nxt
    return cur  # [64, 128]


def _center(W):
    d = W.shape[1]
    return (W.astype(np.float64) @ (np.eye(d) - 1.0 / d)).astype(np.float32)


def _split(a):
    hi = a.astype(F16)
    lo = (a.astype(np.float32) - hi.astype(np.float32)).astype(F16)
    return hi, lo


def _lrelu(x):
    return np.where(x > 0, x, SLOPE * x).astype(np.float32)


def _numpy_forward(inp):
    h = np.asarray(inp["x"], np.float32)
    for i in range(7):
        W = np.asarray(inp[f"W{i+1}"], np.float32)
        g = np.asarray(inp[f"g{i+1}"], np.float32)
        b = np.asarray(inp[f"bt{i+1}"], np.float32)
        h = h @ W
        m = h.mean(-1, keepdims=True)
        v = np.square(h - m).mean(-1, keepdims=True)
        h = (h - m) / np.sqrt(v + EPS) * g + b
        h = _lrelu(h)
    return (h @ np.asarray(inp["W8"], np.float32)
            + np.asarray(inp["b8"], np.float32)).astype(np.float32)


# ---------------------------------------------------------- walrus patches --


def _split_multi_waits(nc):
    """Walrus build limit: <=1 sync wait per instruction. Hoist extras onto
    same-engine NOPs inserted just before the instruction."""
    import concourse.mybir as mybir
    import bass_rust
    cnt = 0
    for f in nc.m.functions:
        for blk in f.blocks:
            newlist = []
            for inst in blk.instructions:
                si = inst.sync_info
                waits = list(si.on_wait) if si is not None and si.on_wait else []
                if len(waits) > 1:
                    for w in waits[:-1]:
                        nop = mybir.InstNoOp(name=f"waitnop_{cnt}", ins=[], outs=[])
                        cnt += 1
                        nop.engine = inst.engine
                        nop.sync_info = bass_rust.SyncInfo(on_wait=[w], on_update=[])
                        newlist.append(nop)
                    inst.sync_info = bass_rust.SyncInfo(
                        on_wait=[waits[-1]], on_update=list(si.on_update))
                newlist.append(inst)
            blk.instructions = newlist
    return cnt


def _patch_tile_drain():
    import bass_rust
    from concourse.tile import TileContext as _TC
    from concourse.vector_clock import ScopedClock

    def _patched_drain(self, tick_clock, wait_clock):
        probe = self.nc.sync.nop()
        wait_clock.add_sem_waits(probe.ins,
                                 ScopedClock({None: tick_clock.global_clock}))
        si = probe.ins.sync_info
        waits = list(si.on_wait) if si is not None else []
        upd = list(si.on_update) if si is not None else []
        probe.ins.sync_info = bass_rust.SyncInfo(on_wait=waits[:1], on_update=upd)
        for w in waits[1:]:
            nop = self.nc.sync.nop()
            nop.ins.sync_info = bass_rust.SyncInfo(on_wait=[w], on_update=[])
        self.nc.sync.drain()
        self.nc.all_engine_barrier()
        assert self.sems is not None
        popped = self.nc._tile_sem_poison_stack.pop()
        assert popped is self._sem_poison
        self.nc.clear_and_free_semaphores(list(self.sems.allocated().values()))
        self.nc.all_engine_barrier()

    _TC._drain_and_barrier = _patched_drain


# ---------------------------------------------------------------- program ---


def _build_consts(inp):
    Wc = [_center(np.asarray(inp[f"W{l}"], np.float32)) for l in range(1, 8)]
    cols = {}
    for l in range(1, 8):
        S = _build_stationary(l, Wc[l - 1])
        hi, lo = _split(S)
        cols[f"s{l}h"] = hi
        cols[f"s{l}l"] = lo
    cols["s8"] = _build_s8(np.asarray(inp["W8"], np.float32)).astype(F16)
    order = sorted(cols.keys())
    offs, total = {}, 0
    for k in order:
        offs[k] = total
        total += cols[k].shape[1]
    wpack = np.zeros((128, total), F16)
    for k in order:
        wpack[:, offs[k]:offs[k] + cols[k].shape[1]] = cols[k]
    vpack = np.concatenate([_build_v6(), _build_v7()], axis=1)  # [128,128] f32
    return wpack, offs


def _build_program(offs, wpack_cols, b8_val):
    import concourse.bass as bass
    import concourse.mybir as mybir
    from concourse.tile import TileContext
    from contextlib import ExitStack

    _patch_tile_drain()

    f16, f32 = mybir.dt.float16, mybir.dt.float32
    AF = mybir.ActivationFunctionType
    OP = mybir.AluOpType

    nc = bass.Bass(trn_type="TRN2", num_swdge_queues=4)
    xhi_d = nc.dram_tensor("xhi", [128, RPC], f16, kind="ExternalInput")
    xlo_d = nc.dram_tensor("xlo", [128, RPC], f16, kind="ExternalInput")
    wp_d = nc.dram_tensor("wpack", [128, wpack_cols], f16, kind="ExternalInput")
    vp_d = nc.dram_tensor("vpack", [128, 128], f32, kind="ExternalInput")
    out_d = nc.dram_tensor("out", [64, N_ST * 128], f32, kind="ExternalOutput")

    with TileContext(nc) as tc:
        with ExitStack() as ctx:
            const = ctx.enter_context(tc.tile_pool(name="const", bufs=1))
            wp = const.tile([128, wpack_cols], f16)
            nc.sync.dma_start(wp[:, :], wp_d[:, :])
            vpk = const.tile([128, 128], f32)
            nc.sync.dma_start(vpk[:, :], vp_d[:, :])

            def W(name, k0, k1, m0, m1):
                o = offs[name]
                return wp[k0:k1, o + m0:o + m1]

            xp = ctx.enter_context(tc.tile_pool(name="xp", bufs=2))
            ap = ctx.enter_context(tc.tile_pool(name="ap", bufs=2))
            afp = ctx.enter_context(tc.tile_pool(name="afp", bufs=3))
            sqp = ctx.enter_context(tc.tile_pool(name="sqp", bufs=2))
            fin = ctx.enter_context(tc.tile_pool(name="fin", bufs=2 * N_ST))
            pA = ctx.enter_context(tc.tile_pool(name="pA", bufs=2, space="PSUM"))
            pB = ctx.enter_context(tc.tile_pool(name="pB", bufs=3, space="PSUM"))

            def eng(c):
                return {"s": nc.scalar, "v": nc.vector, "g": nc.gpsimd}[c]

            def drain(l, u, wlen, col0, ah_t, al_t):
                """Prelu + hi/lo split of psum window u[:, :wlen] into
                a{l}h/a{l}l at cols [col0, col0+wlen)."""
                if l == 7:
                    nc.scalar.activation(ah_t[:, col0:col0 + wlen],
                                         u[:, :wlen], AF.Prelu,
                                         bias=0.0, scale=1.0, alpha=SLOPE)
                    return
                af = afp.tile([128, 1024], f32, name="af", tag="af")
                afv = af[:, :wlen]
                if PRELU_ENG[l] == 's':
                    nc.scalar.activation(afv, u[:, :wlen], AF.Prelu,
                                         bias=0.0, scale=1.0, alpha=SLOPE)
                else:
                    # DVE 2-instr Prelu (stt cannot read 2 PSUM operands):
                    # t = 0.2*u (PSUM->SBUF), af = max(u, t)
                    tt = afp.tile([128, 1024], f32, name="t02", tag="t02")
                    nc.vector.tensor_scalar(tt[:, :wlen], u[:, :wlen], SLOPE,
                                            None, OP.mult)
                    nc.vector.tensor_tensor(afv, u[:, :wlen], tt[:, :wlen],
                                            OP.max)
                hv = ah_t[:, col0:col0 + wlen]
                nc.vector.tensor_copy(hv, afv)
                eng(LO_ENG[l]).tensor_tensor(al_t[:, col0:col0 + wlen],
                                             afv, hv, OP.subtract)

            ysbs, e8sbs = [], []

            for st in range(N_ST):
                x0 = st * R_ST
                xh = xp.tile([128, R_ST], f16, name="xh", tag="xh")
                nc.sync.dma_start(xh[:, :], xhi_d[:, x0:x0 + R_ST])
                xl = xp.tile([128, R_ST], f16, name="xl", tag="xl")
                nc.sync.dma_start(xl[:, :], xlo_d[:, x0:x0 + R_ST])

                prev_h, prev_l = xh, xl
                s6 = s7 = a7 = None
                for l in range(1, 8):
                    n = N_OUT[l]
                    ah_t = ap.tile([128, n], f16, name=f"a{l}h", tag=f"a{l}h")
                    al_t = None
                    if l < 7:
                        al_t = ap.tile([128, n], f16, name=f"a{l}l",
                                       tag=f"a{l}l")
                    for w0 in WINDOWS[l]:
                        wlen = min(1024, n - w0)
                        if l >= 5:
                            u = pB.tile([128, 512], f32, name="uB", tag="uB",
                                        bufs=2)
                        else:
                            u = pA.tile([128, 1024], f32, name="uA", tag="uA")
                        w1 = w0 + wlen
                        for t in range(3):
                            suf = "h" if t < 2 else "l"
                            rh = prev_h if t != 1 else prev_l
                            for sl in LAYER_SLOTS[l]:
                                o0 = max(w0, sl["oc"][0])
                                o1 = min(w1, sl["oc"][1])
                                if o0 >= o1:
                                    continue
                                k0, k1 = sl["kr"]
                                m0, m1 = sl["mr"]
                                lhsT = W(f"s{l}{suf}", k0, k1, m0, m1)
                                for o in range(o0, o1, 512):
                                    e = min(o + 512, o1)
                                    r0 = sl["rhs"][0] + (o - sl["oc"][0])
                                    nc.tensor.matmul(
                                        u[m0:m1, o - w0:e - w0], lhsT,
                                        rh[k0:k1, r0:r0 + (e - o)],
                                        start=(t == 0), stop=(t == 2),
                                        tile_position=sl["tp"])
                        if l == 6:
                            s6 = sqp.tile([128, 256], f32, name="s6", tag="s6")
                            nc.scalar.activation(s6[:, :], u[:, :256],
                                                 AF.Square)
                        if l == 7:
                            s7 = sqp.tile([128, 128], f32, name="s7", tag="s7")
                            nc.scalar.activation(s7[:, :], u[:, :128],
                                                 AF.Square)
                        drain(l, u, wlen, w0, ah_t, al_t)
                    prev_h, prev_l = ah_t, al_t
                a7 = prev_h

                # variances (fp32 matmuls) + y
                v6t = pB.tile([64, 128], f32, name="v6t", tag="vv", bufs=2)
                nc.tensor.matmul(v6t[0:32, :], vpk[:, 0:32], s6[:, 0:128],
                                 start=True, stop=True, tile_position=(0, 0))
                nc.tensor.matmul(v6t[32:64, :], vpk[:, 32:64], s6[:, 128:256],
                                 start=True, stop=True, tile_position=(0, 32))
                v7t = pB.tile([64, 128], f32, name="v7t", tag="vv", bufs=2)
                nc.tensor.matmul(v7t[:, :], vpk[:, 64:128], s7[:, :],
                                 start=True, stop=True)
                yt = pB.tile([64, 128], f32, name="yt", tag="vv", bufs=2)
                nc.tensor.matmul(yt[:, :], W("s8", 0, 128, 0, 64), a7[:, :],
                                 start=True, stop=True)

                v7sb = fin.tile([64, 128], f32, name="v7sb", tag="v7sb",
                                bufs=2)
                nc.scalar.copy(v7sb[:, :], v7t[:, :])
                e8 = fin.tile([64, 128], f32, name="e8", tag="e8", bufs=N_ST)
                nc.vector.scalar_tensor_tensor(e8[:, :], v6t[:, :], EPS,
                                               v7sb[:, :], OP.mult, OP.add)
                ysb = fin.tile([64, 128], f32, name="ysb", tag="ysb",
                               bufs=N_ST)
                nc.vector.tensor_copy(ysb[:, :], yt[:, :])
                ysbs.append(ysb)
                e8sbs.append(e8)

            for st in range(N_ST):
                sq = fin.tile([64, 128], f32, name="sq", tag="sq", bufs=2)
                nc.scalar.activation(sq[:, :], e8sbs[st][:, :], AF.Sqrt)
                rinv = fin.tile([64, 128], f32, name="rinv", tag="rinv", bufs=2)
                nc.vector.reciprocal(rinv[:, :], sq[:, :])
                osb = fin.tile([64, 128], f32, name="osb", tag="osb", bufs=2)
                nc.vector.tensor_tensor(osb[:, :], ysbs[st][:, :], rinv[:, :],
                                        OP.mult)
                nc.vector.tensor_scalar(osb[:, :], osb[:, :], b8_val,
                                        None, OP.add)
                nc.sync.dma_start(out_d[:, st * 128:(st + 1) * 128], osb[:, :])

    _split_multi_waits(nc)
    return nc


def kernel(**inputs):
    for l in range(1, 8):
        if np.abs(np.asarray(inputs[f"bt{l}"], np.float32)).max() > 0:
            return _numpy_forward(inputs)
        g = np.asarray(inputs[f"g{l}"], np.float32)
        if np.abs(g - 1.0).max() > 0:
            return _numpy_forward(inputs)

    wpack, offs = _build_consts(inputs)
    vpack = np.ascontiguousarray(
        np.concatenate([_build_v6(), _build_v7()], axis=1), dtype=np.float32)
    b8 = float(np.asarray(inputs["b8"], np.float32).reshape(-1)[0])

    x = np.asarray(inputs["x"], np.float32)
    xT = np.ascontiguousarray(x.T)               # [128, 524288]
    xhi = xT.astype(F16)
    xlo = (xT - xhi.astype(np.float32)).astype(F16)

    nc = _build_program(offs, wpack.shape[1], b8)

    in_maps = []
    for c in range(N_CORES):
        s = slice(c * RPC, (c + 1) * RPC)
        in_maps.append({
            "xhi": np.ascontiguousarray(xhi[:, s]),
            "xlo": np.ascontiguousarray(xlo[:, s]),
            "wpack": wpack, "vpack": vpack,
        })

    from concourse.bass_utils import run_bass_kernel_spmd
    res = run_bass_kernel_spmd(nc, in_maps, core_ids=list(range(N_CORES)))

    rm7 = _rowmap7()                             # [64, 128]
    perm = (np.arange(N_ST)[None, :, None] * R_ST
            + rm7[:, None, :]).reshape(-1)       # [64, N_ST, 128] -> flat
    out = np.empty((ROWS, 1), np.float32)
    for c in range(N_CORES):
        o = np.asarray(res.results[c]["out"], np.float32).reshape(-1)
        blk = np.empty(RPC, np.float32)
        blk[perm] = o
        out[c * RPC:(c + 1) * RPC, 0] = blk
    return out
